# revision 22
# baseline (speedup 1.0000x reference)
"""Trainium2 Bass kernel for nn_ConvGraphSelfLoop.

out = where(any(adj>=0, axes -1,-2), relu(features @ W + b), features)

Strategy (device does the GEMM, host does layout + select):
  - A vertex is "valid" iff any adjacency entry >= 0. Invalid vertices
    pass their input features through untouched — the host writes those
    directly from the fp32 input, so the device only transforms valid
    vertices (49248 of 65536 for the fixed seed).
  - Host compacts the valid vertices, casts to fp16, transposes to
    xT [F, n] and splits them evenly across 8 cores. Capacity is 6144
    tokens/core (12 blocks of 512) = 49152 total; the small remainder
    beyond capacity is computed on the host in fp32 — correctness never
    depends on the capacity bound.
  - Device computes outT = relu(W^T @ xT + b) in transposed space:
      * W [F, U] has the contraction dim on partitions, so W chunks are
        the stationary operand — no PE transposes.
      * bias rides the ACT eviction (per-partition bias operand).
      * fp16 operands: 1 cyc/row PE rate, half the DMA of fp32.
  - Schedule is built to keep the PE at its 213ns/512-row streaming
    rate from first to last matmul:
      * 9 warm-up matmuls on a memset tile ramp the PE p-state during
        the DMA cold start (PE would otherwise idle ~12us — data-ready
        is gated by DMA-completion semaphores that lag the transfer by
        ~2us — and then run its first ~3us at the slow p-state).
      * superblock 0 (512 tokens) runs f-outer so it only needs the
        first W slab (two parallel 128KB halves) + one 128KB x chunk
        to start, and consumes them at the 2-queue DMA delivery pace.
      * W slabs and x chunks interleave across the sync+gpsimd DMA
        queues (the only FAST hw queues: ~147 B/ns each; the scalar
        engine's queue measured 63-76 B/ns AND is starved while the
        other queues are active, so it carries nothing but the 4KB
        bias). Outputs split across sync/gpsimd by u parity, with
        each superblock's input slices enqueued ahead of any output
        that could head-of-line block the FIFO.
      * remaining superblocks [512, 512, 2048, 2048, 512] run u-major
        with uniform [128,512] psum tiles (one bank, 8-buf ring) so
        evictions stay staggered and the tail superblock drains fast.
        Small supers lead so the big 4MB slabs face late deadlines.
      * the Tile scheduler reorders DMA issues (hoists always-ready
        input loads over eviction-gated output stores), so output r
        rings are sized (16 for 512-wide, 8 for 2048-wide supers) such
        that no eviction — and via the psum ring, the PE — ever waits
        on an output DMA that the scheduler deferred.
"""
import numpy as np
import concourse.bass as bass
import concourse.bacc as bacc
import concourse.mybir as mybir
import concourse.tile as tile
from concourse.bass_utils import run_bass_kernel_spmd

B, V, E, NN = 4, 16384, 4, 32
F, U = 1024, 1024
NCORES = 8
P = 128
BLK = 512                    # tokens per psum bank / matmul free dim
NBLK = 12                    # token-blocks per core (capacity 6144)
CAP = NBLK * BLK             # 6144 tokens per core
SUPERS = [(0, 512), (512, 512), (1024, 512), (1536, 2048),
          (3584, 2048), (5632, 512)]
CF = F // P                  # 8 contraction chunks
CU = U // P                  # 8 output-partition chunks
NDUM = 9                     # PE p-state warm-up matmuls

f32 = mybir.dt.float32
f16 = mybir.dt.float16
AF = mybir.ActivationFunctionType


def _build():
    nc = bacc.Bacc("TRN2", target_bir_lowering=False, debug=False,
                   num_devices=NCORES)
    xt_d = nc.dram_tensor("xt", [F, CAP], f16, kind="ExternalInput")
    w_d = nc.dram_tensor("weight", [F, U], f16, kind="ExternalInput")
    bias_d = nc.dram_tensor("bias", [P, CU], f32, kind="ExternalInput")
    out_d = nc.dram_tensor("outT", [U, CAP], f16, kind="ExternalOutput")

    with tile.TileContext(nc) as tc:
        with tc.tile_pool(name="const", bufs=1) as const, \
             tc.tile_pool(name="xp", bufs=1) as xp, \
             tc.tile_pool(name="op", bufs=3) as op, \
             tc.tile_pool(name="psp", bufs=8, space="PSUM") as psp:

            # ---- warm-up tile (vector engine is idle until outputs) ----
            xdum = const.tile([P, BLK], f16)
            nc.vector.memset(xdum[:], 0.25)

            # ---- resident constants + all input DMAs ----
            # w_sb[:, f*U + j] = W[f*P + p, j]  (slab f = W rows f*P..)
            w_sb = const.tile([P, CF * U], f16)
            bias_sb = const.tile([P, CU], f32)
            nc.scalar.dma_start(bias_sb[:], bias_d.ap())

            xs_tiles = []
            for i, (off, W) in enumerate(SUPERS):
                xs = xp.tile([P, CF * W], f16, tag=f"xs{i}", name=f"xs{i}")
                xs_tiles.append(xs)

            def load_xs(si, f, eng):
                off, W = SUPERS[si]
                eng.dma_start(xs_tiles[si][:, f * W:(f + 1) * W],
                              xt_d.ap()[f * P:(f + 1) * P, off:off + W])

            # Interleave W slab halves with superblock-0 chunk f on two
            # queues: each W slab splits [0:512]/[512:1024] across
            # sync/gpsimd so the first f-iteration's operands land ~2us
            # sooner (u0-3 only need half-a, u4-7 half-b — subtile deps
            # release per half). Superblock 0's f-outer loop consumes
            # in exactly this order.
            H = U // 2
            for f in range(CF):
                nc.sync.dma_start(w_sb[:, f * U:f * U + H],
                                  w_d.ap()[f * P:(f + 1) * P, 0:H])
                nc.gpsimd.dma_start(w_sb[:, f * U + H:(f + 1) * U],
                                    w_d.ap()[f * P:(f + 1) * P, H:U])
                load_xs(0, f, nc.sync if f % 2 == 0 else nc.gpsimd)
            # Lookahead: superblock 1 loads while 0 computes. Later
            # superblocks' inputs are interleaved with earlier
            # superblocks' outputs (input first in each pair) so the
            # eviction-paced output transfers never head-of-line block
            # input delivery.
            for f in range(CF):
                load_xs(1, f, nc.sync if f % 2 == 0 else nc.gpsimd)

            # ---- PE p-state warm-up during the DMA cold start ----
            dums = [psp.tile([P, BLK], f32, tag="ps", name=f"dum{i}")
                    for i in range(CU)]
            for i in range(NDUM):
                nc.tensor.matmul(dums[i % CU][:], xdum[:, 0:P], xdum[:],
                                 start=True, stop=True)

            def lhsT(f, u):
                return w_sb[:, f * U + u * P: f * U + (u + 1) * P]

            def store_r(si, u, r_ap):
                # Outputs ride the same two fast queues, split by u
                # parity. Issue position (relative to input slices) is
                # chosen by emission order below. The final superblock
                # goes entirely to sync (its hw queue drains faster at
                # the tail) and superblock 4 entirely to gpsimd, so
                # gpsimd's slower-draining queue finishes early.
                off, W = SUPERS[si]
                if si == len(SUPERS) - 1:
                    eng = nc.sync
                elif si == len(SUPERS) - 2:
                    eng = nc.gpsimd
                else:
                    eng = nc.sync if u % 2 == 0 else nc.gpsimd
                eng.dma_start(out_d.ap()[u * P:(u + 1) * P, off:off + W],
                              r_ap)

            # ---- superblock 0: f-outer (start on first W slab + x chunk) ----
            off0, W0 = SUPERS[0]
            ps0 = [psp.tile([P, BLK], f32, tag="ps", name=f"ps0_{u}")
                   for u in range(CU)]
            for f in range(CF):
                for u in range(CU):
                    nc.tensor.matmul(ps0[u][:], lhsT(f, u),
                                     xs_tiles[0][:, f * W0:(f + 1) * W0],
                                     start=(f == 0), stop=(f == CF - 1))
            r0 = op.tile([P, CU * BLK], f16, tag="r0", bufs=1)
            for u in range(CU):
                nc.scalar.activation(r0[:, u * BLK:(u + 1) * BLK], ps0[u][:],
                                     AF.Relu, bias=bias_sb[:, u:u + 1])

            # ---- superblocks 1..: u-major, per-512 psum tiles ----
            # Emission order per queue: superblock si's outputs are
            # enqueued BEFORE superblock si+2's input slices. Outputs
            # transfer just-in-time as evictions land (brief queue
            # idle-waits are fine); the input slices behind them still
            # arrive a full superblock ahead of their deadline. The
            # reverse order (v3) let an output's r-slot reuse chain
            # block the PE for 7.7us behind 2MB of early input.
            def compute_super(si):
                off, W = SUPERS[si]
                nb = W // BLK
                # rs ring is 16 deep: the Tile scheduler hoists future
                # input loads ahead of output stores in each DMA queue,
                # so an output can transfer a full superblock late — a
                # ring of 8 would tie a 512-super's eviction (and via
                # the psum ring, the PE) to that late DMA.
                tag = "rw" if nb > 1 else "rs"
                rs = []
                for u in range(CU):
                    r = op.tile([P, W], f16, tag=tag, name=f"r{si}_{u}",
                                bufs=8 if nb > 1 else 16)
                    for b in range(nb):
                        ps = psp.tile([P, BLK], f32, tag="ps",
                                      name=f"ps{si}_{u}_{b}")
                        for f in range(CF):
                            nc.tensor.matmul(
                                ps[:],
                                lhsT(f, u),
                                xs_tiles[si][:, f * W + b * BLK:
                                             f * W + (b + 1) * BLK],
                                start=(f == 0), stop=(f == CF - 1))
                        nc.scalar.activation(r[:, b * BLK:(b + 1) * BLK],
                                             ps[:], AF.Relu,
                                             bias=bias_sb[:, u:u + 1])
                    rs.append(r)
                return rs

            def emit_pair(out_si, r_aps, in_si):
                # Per step k: input slice f=k first, then output u=k —
                # same queue parity, so each queue's FIFO alternates
                # [in, out, in, out, ...] and input slabs flow during
                # the idle-waits on not-yet-evicted outputs.
                for k in range(CF):
                    if in_si is not None:
                        load_xs(in_si, k,
                                nc.sync if k % 2 == 0 else nc.gpsimd)
                    store_r(out_si, k, r_aps[k])

            emit_pair(0, [r0[:, u * BLK:(u + 1) * BLK] for u in range(CU)],
                      2)                  # s2 inputs with s0 outputs
            rs1 = compute_super(1)
            emit_pair(1, [r[:] for r in rs1], 3)   # s3 inputs, s1 outs
            rs2 = compute_super(2)
            emit_pair(2, [r[:] for r in rs2], 4)   # s4 inputs, s2 outs
            for f in range(CF):           # superblock 5 inputs
                load_xs(5, f, nc.sync if f % 2 == 0 else nc.gpsimd)
            for si in (3, 4, 5):
                rs = compute_super(si)
                emit_pair(si, [r[:] for r in rs], None)

    nc.compile()
    return nc


_nc_cache = None


def _get_nc():
    global _nc_cache
    if _nc_cache is None:
        _nc_cache = _build()
    return _nc_cache


def _preprocess(inputs):
    """Host-side: mask, compaction, fp16 transpose, per-core split."""
    feats2 = np.asarray(inputs["features"], dtype=np.float32).reshape(B * V, F)
    adj2 = np.asarray(inputs["adjacency"]).reshape(B * V, E * NN)
    valid = adj2.max(axis=1) >= 0
    idx = np.flatnonzero(valid)
    dev_idx = idx[:NCORES * CAP]          # device-computed valid tokens
    ovf_idx = idx[NCORES * CAP:]          # host fallback (small remainder)

    w16 = np.ascontiguousarray(inputs["kernel"], dtype=np.float16)
    bias = np.asarray(inputs["bias"], dtype=np.float32).reshape(-1)
    bias_dev = np.ascontiguousarray(bias.reshape(CU, P).T, dtype=np.float32)

    n = dev_idx.size
    counts = [(n + NCORES - 1 - i) // NCORES for i in range(NCORES)]
    starts = np.cumsum([0] + counts)
    in_maps, core_idx = [], []
    for i in range(NCORES):
        ci = dev_idx[starts[i]:starts[i + 1]]
        core_idx.append(ci)
        xti = np.zeros((F, CAP), dtype=np.float16)
        if ci.size:
            xti[:, :ci.size] = feats2[ci].T.astype(np.float16)
        in_maps.append({"xt": xti, "weight": w16, "bias": bias_dev})
    return feats2, valid, core_idx, ovf_idx, in_maps


def _make_in_maps(inputs):
    return _preprocess(inputs)[4]


def kernel(adjacency, features, kernel, bias):
    nc = _get_nc()
    inputs = {"adjacency": adjacency, "features": features,
              "kernel": kernel, "bias": bias}
    feats2, valid, core_idx, ovf_idx, in_maps = _preprocess(inputs)
    res = run_bass_kernel_spmd(nc, in_maps, list(range(NCORES)))

    out = np.empty((B * V, U), dtype=np.float32)
    out[~valid] = feats2[~valid]
    for i in range(NCORES):
        ci = core_idx[i]
        if ci.size:
            oT = res.results[i]["outT"]
            out[ci] = oT[:, :ci.size].T.astype(np.float32)
    if ovf_idx.size:
        w32 = np.asarray(kernel, dtype=np.float32)
        b32 = np.asarray(bias, dtype=np.float32).reshape(-1)
        out[ovf_idx] = np.maximum(feats2[ovf_idx] @ w32 + b32, 0.0)
    return out.reshape(B, V, U)
